# revision 1
# baseline (speedup 1.0000x reference)
"""Trainium2 Bass kernel for AttLayer pooling (B=32, T=2048, D=1024, H=5).

Math (equivalent to reference up to exact cancellation of the softmax
normalization): since |tanh| <= 1, scores s[b,t] are bounded by ||uw||_1, so
exp needs no max-subtraction, and the masked renormalization cancels the
softmax denominator:

    out[b,:] = sum_t x[b,t,:] * g[b,t] / sum_t g[b,t]
    g[b,t]   = exp(s[b,t] + masklog[b,t]),  masklog = 0 or -1e30
    s[b,t]   = sum_h tanh( (x @ W)[b,t,h] + fea[b,t]*Wf[h] + bw[h] ) * uw[h]

Everything is tile-local (no cross-T dependency), so the kernel streams x
in a single pass. Data-parallel across batch: 8 cores x 4 batches each.

Per 128-t tile of x [128, 1024]:
  - PE transposes x chunks (f32r, is_transpose) -> psum -> DVE copy -> xT sbuf
  - scores: psum[5, T_GRP] += W_chunk.T @ xT_chunk   (f32r, N=T_GRP)
            + fea part via K=1 matmul (lhsT=Wf [1,5], rhs=fea row)
  - ACT tanh(scores + bw) -> tanh_b rows 0..4; masklog precomputed in row 5
  - uw matmul per tile: lhsT = tanh_b[:, chunk] [6,128], rhs = uw_aug [6,1]
    -> s' column [128,1] in psum (mask fold: uw_aug[5]=1, row5 = masklog)
  - ACT exp -> g [128,1] f32r
  - num matmuls: psum[1, 1025] += g.T @ [x | ones]  (f32r; col 1024 = den)
Final per batch: out = num * reciprocal(den), DMA out.
"""

import sys

sys.path.insert(0, "/opt/trn_rl_repo")

import numpy as np

import concourse.bass as bass
import concourse.mybir as mybir
import concourse.tile as tile
from concourse import bacc
from concourse.masks import make_identity
from concourse import bass_isa

F32 = mybir.dt.float32
F32R = mybir.dt.float32r
BF16 = mybir.dt.bfloat16
U8 = mybir.dt.uint8
AF = mybir.ActivationFunctionType

P = 128          # partitions / t-tile size
D = 1024         # feature dim
H = 5            # attention hidden dim
NCHUNK = D // P  # 8 d-chunks per tile


def build_kernel(b_shard: int, T: int, t_grp: int = 512, dma_grp: int = 512):
    """Build the per-core Bass program.

    b_shard: batches per core; T: sequence length; t_grp: t per compute
    group (multiple of 128, <= 512); dma_grp: t per DMA chunk (multiple of
    t_grp).
    """
    assert t_grp % P == 0 and T % dma_grp == 0 and dma_grp % t_grp == 0
    jg = t_grp // P            # tiles per compute group
    jd = dma_grp // P          # tiles per DMA chunk
    n_dma = T // dma_grp
    grp_per_dma = dma_grp // t_grp

    nc = bacc.Bacc(None)

    x_temp = nc.dram_tensor("x_temp", [b_shard, T, D], F32R, kind="ExternalInput")
    x_fea = nc.dram_tensor("x_fea", [b_shard, T], F32R, kind="ExternalInput")
    mask = nc.dram_tensor("mask", [b_shard, T], U8, kind="ExternalInput")
    W_temp = nc.dram_tensor("W_temp", [D, H], F32, kind="ExternalInput")
    W_fea = nc.dram_tensor("W_fea", [1, H], F32R, kind="ExternalInput")
    bw = nc.dram_tensor("bw", [H], F32, kind="ExternalInput")
    uw = nc.dram_tensor("uw", [H], F32, kind="ExternalInput")
    out = nc.dram_tensor("out", [b_shard, D], F32, kind="ExternalOutput")

    with tile.TileContext(nc) as tc:
        with (
            tc.tile_pool(name="consts", bufs=1) as consts,
            tc.tile_pool(name="xpool", bufs=3) as xpool,
            tc.tile_pool(name="xtpool", bufs=3) as xtpool,
            tc.tile_pool(name="rows", bufs=2) as rows,
            tc.tile_pool(name="small", bufs=2) as small,
            tc.tile_pool(name="tp_ps", bufs=3, space="PSUM") as tp_ps,
            tc.tile_pool(name="sc_ps", bufs=2, space="PSUM") as sc_ps,
            tc.tile_pool(name="g_ps", bufs=1, space="PSUM") as g_ps,
            tc.tile_pool(name="num_ps", bufs=1, space="PSUM") as num_ps,
        ):
            # ---- constants ----
            # Transposes + scores matmuls run in bf16 (x cast on GpSimd);
            # num matmuls stay f32r on the raw DMA'd x (exact-ish).
            ident = consts.tile([P, P], BF16)
            make_identity(nc, ident[:])
            w_f = consts.tile([P, NCHUNK, H], F32)
            nc.sync.dma_start(w_f[:], W_temp.rearrange("(c p) h -> p c h", p=P))
            w_sb = consts.tile([P, NCHUNK, H], BF16)
            nc.vector.tensor_copy(w_sb[:], w_f[:])
            wf_sb = consts.tile([1, H], F32R)
            nc.sync.dma_start(wf_sb[:], W_fea[:])
            bw_sb = consts.tile([H, 1], F32)
            nc.sync.dma_start(bw_sb[:], bw[:, None])
            # uw_aug = [uw; 1.0]: memset whole tile to 1.0, DMA uw over rows 0..4
            # (engine ops cannot write at base partition 5, DMA can overwrite 0..4)
            uwa_f = consts.tile([H + 1, 2], F32)
            nc.vector.memset(uwa_f[:], 1.0)
            nc.sync.dma_start(uwa_f[:H, 0:1], uw[:, None])
            nc.sync.dma_start(uwa_f[:H, 1:2], uw[:, None])
            uwa_sb = consts.tile([H + 1, 2], F32R)
            nc.vector.tensor_copy(uwa_sb[:], uwa_f[:])

            for b in range(b_shard):
                # ---- per-batch rows ----
                fea_sb = rows.tile([1, T], F32R, tag="fea")
                nc.sync.dma_start(fea_sb[:], x_fea[b : b + 1, :])
                mask_f = rows.tile([1, T], F32, tag="maskf")
                nc.gpsimd.dma_start(mask_f[:], mask[b : b + 1, :])  # u8 -> f32 cast
                masklog = rows.tile([1, T], F32R, tag="masklog")
                nc.scalar.activation(
                    masklog[:], mask_f[:], AF.Copy, scale=1.0e30, bias=-1.0e30
                )
                # tanh_b rows 0..4 = tanh(scores) written per group;
                # row 5 = masklog = mask*1e30 - 1e30  (0 or -1e30).
                # SBUF->SBUF DMA: engines can't write at base partition 5.
                tanh_b = rows.tile([H + 1, T], F32R, tag="tanhb")
                nc.sync.dma_start(tanh_b[H : H + 1, :], masklog[:])

                nm = num_ps.tile([1, D], F32, tag="num")
                n_tiles = T // P
                g_sb = rows.tile([P, n_tiles], F32R, tag="gsb")

                # num matmuls for group g are emitted while group g+1's
                # scores run (one-group software pipeline), so the PE never
                # stalls on the exp(g) -> num-LDW dependency.
                pending = None

                def emit_num(p):
                    g_, x3_, gi_ = p
                    for j_ in range(jg):
                        tt_ = g_ * jg + j_
                        nc.tensor.matmul(
                            nm[:, 0:512],
                            g_sb[:, tt_ : tt_ + 1],
                            x3_[:, gi_ * jg + j_, 0:512],
                            start=(tt_ == 0),
                            stop=(tt_ == n_tiles - 1),
                        )
                        nc.tensor.matmul(
                            nm[:, 512:1024],
                            g_sb[:, tt_ : tt_ + 1],
                            x3_[:, gi_ * jg + j_, 512:1024],
                            start=(tt_ == 0),
                            stop=(tt_ == n_tiles - 1),
                        )

                for di in range(n_dma):
                    x3 = xpool.tile([P, jd, D], F32R, tag="x")
                    nc.sync.dma_start(
                        x3[:],
                        x_temp[b, di * dma_grp : (di + 1) * dma_grp, :].rearrange(
                            "(j p) d -> p j d", p=P
                        ),
                    )
                    for gi in range(grp_per_dma):
                        g = di * grp_per_dma + gi   # group index within batch
                        t0 = g * t_grp
                        # bf16 copy of this group's x for transposes/scores
                        xb = xtpool.tile([P, jg, D], BF16, tag="xb")
                        for j in range(jg):
                            src = x3[:, gi * jg + j, :].bitcast(F32)
                            if j % 4 == 3:
                                nc.scalar.copy(xb[:, j, :], src)
                            else:
                                nc.vector.tensor_copy(xb[:, j, :], src)
                        sc = sc_ps.tile([H, t_grp], F32, tag="sc")
                        # fea part: [5, t_grp] = Wf.T @ fea_row (K=1), starts accum
                        nc.tensor.matmul(
                            sc[:],
                            wf_sb[:],
                            fea_sb[:, t0 : t0 + t_grp],
                            start=True,
                            stop=False,
                        )
                        if pending is not None:
                            emit_num(pending)
                            pending = None
                        for cp in range(NCHUNK // 2):
                            tp = tp_ps.tile([P, 2, t_grp], BF16, tag="tp")
                            for c2 in range(2):
                                c = cp * 2 + c2
                                for j in range(jg):
                                    nc.tensor.transpose(
                                        tp[:, c2, j * P : (j + 1) * P],
                                        xb[:, j, c * P : (c + 1) * P],
                                        ident[:],
                                    )
                            xt = xtpool.tile([P, 2, t_grp], BF16, tag="xt")
                            nc.vector.tensor_copy(xt[:], tp[:])
                            for c2 in range(2):
                                c = cp * 2 + c2
                                nc.tensor.matmul(
                                    sc[:],
                                    w_sb[:, c, :],
                                    xt[:, c2, :],
                                    start=False,
                                    stop=(c == NCHUNK - 1),
                                )
                        # tanh(sc + bw) -> tanh_b rows 0..4
                        nc.scalar.activation(
                            tanh_b[:H, t0 : t0 + t_grp], sc[:], AF.Tanh, bias=bw_sb[:]
                        )
                        # uw matmuls: one [128,1] s' column per tile
                        gp = g_ps.tile([P, jg, 2], F32, tag="g")
                        for j in range(jg):
                            nc.tensor.matmul(
                                gp[:, j, :],
                                tanh_b[:, t0 + j * P : t0 + (j + 1) * P],
                                uwa_sb[:],
                                start=True,
                                stop=True,
                            )
                        nc.scalar.activation(
                            g_sb[:, g * jg : (g + 1) * jg], gp[:, :, 0], AF.Exp
                        )
                        pending = (g, x3, gi)

                if pending is not None:
                    emit_num(pending)
                    pending = None

                # den = sum of g: DVE free-reduce then GpSimd partition reduce
                gcs = small.tile([P, 1], F32, tag="gcs")
                nc.vector.tensor_reduce(
                    gcs[:],
                    g_sb[:].bitcast(F32),
                    axis=mybir.AxisListType.X,
                    op=mybir.AluOpType.add,
                )
                den_sb = small.tile([P, 1], F32, tag="densb")
                nc.gpsimd.partition_all_reduce(
                    den_sb[:], gcs[:], channels=P, reduce_op=bass_isa.ReduceOp.add
                )
                inv = small.tile([1, 1], F32, tag="inv")
                nc.vector.reciprocal(inv[:], den_sb[0:1, :])
                o_sb = small.tile([1, D], F32, tag="osb")
                nc.vector.tensor_scalar_mul(o_sb[:], nm[:, :D], inv[:])
                nc.sync.dma_start(out[b : b + 1, :], o_sb[:])

    nc.finalize()
    return nc


_NC_CACHE = {}


def _get_nc(b_shard, T):
    key = (b_shard, T)
    if key not in _NC_CACHE:
        _NC_CACHE[key] = build_kernel(b_shard, T)
    return _NC_CACHE[key]


def kernel(x_temp, x_fea, mask, W_temp, W_fea, bw, uw) -> np.ndarray:
    from concourse.bass_utils import run_bass_kernel_spmd

    B, T, D_ = x_temp.shape
    n_cores = 8
    assert B % n_cores == 0
    bs = B // n_cores

    nc = _get_nc(bs, T)

    x_temp = np.ascontiguousarray(x_temp, dtype=np.float32)
    x_fea = np.ascontiguousarray(x_fea, dtype=np.float32)
    mask_u8 = np.ascontiguousarray(mask).view(np.uint8)
    W_temp = np.ascontiguousarray(W_temp, dtype=np.float32)
    W_fea = np.ascontiguousarray(W_fea, dtype=np.float32)
    bw = np.ascontiguousarray(bw, dtype=np.float32)
    uw = np.ascontiguousarray(uw, dtype=np.float32)

    in_maps = []
    for i in range(n_cores):
        in_maps.append(
            {
                "x_temp": x_temp[i * bs : (i + 1) * bs],
                "x_fea": x_fea[i * bs : (i + 1) * bs],
                "mask": mask_u8[i * bs : (i + 1) * bs],
                "W_temp": W_temp,
                "W_fea": W_fea,
                "bw": bw,
                "uw": uw,
            }
        )

    res = run_bass_kernel_spmd(nc, in_maps, core_ids=list(range(n_cores)))
    return np.concatenate([r["out"] for r in res.results], axis=0)



# revision 2
# speedup vs baseline: 1.0289x; 1.0289x over previous
"""Trainium2 Bass kernel for AttLayer pooling (B=32, T=2048, D=1024, H=5). v5

Math identical to baseline kernel.py, with the mask fold restated:
    out[b,:] = sum_t x[b,t,:] * g[b,t] / sum_t g[b,t]
    g[b,t]   = exp(s[b,t] + 100*mask[b,t] - 100)   (masked: exp(s-100) -> 0)
    s[b,t]   = sum_h tanh( (x @ W)[b,t,h] + fea[b,t]*Wf[h] + bw[h] ) * uw[h]

v5 pipeline (on top of v3):
  - mask handled with zero extra ops: raw mask u8 row DMA-cast straight into
    tanh_b row 5, uw_aug row 5 = 100.0, exp bias = -100.0. Removes the
    masklog ACT (whose late mask DMA head-of-line blocked exp/tanh in the
    ACT FIFO and stalled the PE ~7us per batch).
  - batch-0 chunk-0 issued as 4 sub-DMAs so transposes start ~5us in.
"""

import sys

sys.path.insert(0, "/opt/trn_rl_repo")

import numpy as np

import concourse.bass as bass
import concourse.mybir as mybir
import concourse.tile as tile
from concourse import bacc
from concourse.masks import make_identity

F32 = mybir.dt.float32
F32R = mybir.dt.float32r
BF16 = mybir.dt.bfloat16
U8 = mybir.dt.uint8
AF = mybir.ActivationFunctionType

P = 128          # partitions / t-tile size
D = 1024         # feature dim
H = 5            # attention hidden dim
NCHUNK = D // P  # 8 d-chunks per tile

USE_DMA_CAST = True


def build_kernel(b_shard: int, T: int, t_grp: int = 512, dma_grp: int = 1024,
                 use_dma_cast: bool = USE_DMA_CAST):
    assert t_grp % P == 0 and T % dma_grp == 0 and dma_grp % t_grp == 0
    jg = t_grp // P            # tiles per compute group
    jd = dma_grp // P          # tiles per DMA chunk
    n_dma = T // dma_grp
    grp_per_dma = dma_grp // t_grp
    n_tiles = T // P

    nc = bacc.Bacc(None)

    x_temp = nc.dram_tensor("x_temp", [b_shard, T, D], F32, kind="ExternalInput")
    x_fea = nc.dram_tensor("x_fea", [b_shard, T], F32R, kind="ExternalInput")
    mask = nc.dram_tensor("mask", [b_shard, T], U8, kind="ExternalInput")
    W_temp = nc.dram_tensor("W_temp", [D, H], F32, kind="ExternalInput")
    W_fea = nc.dram_tensor("W_fea", [1, H], F32R, kind="ExternalInput")
    bw = nc.dram_tensor("bw", [H], F32, kind="ExternalInput")
    uw = nc.dram_tensor("uw", [H], F32, kind="ExternalInput")
    out = nc.dram_tensor("out", [b_shard, D], F32, kind="ExternalOutput")

    with tile.TileContext(nc) as tc:
        with (
            tc.tile_pool(name="consts", bufs=1) as consts,
            tc.tile_pool(name="xpool", bufs=3) as xpool,
            tc.tile_pool(name="xbpool", bufs=3) as xbpool,
            tc.tile_pool(name="xtpool", bufs=6) as xtpool,
            tc.tile_pool(name="rows", bufs=2) as rows,
            tc.tile_pool(name="small", bufs=2) as small,
            tc.tile_pool(name="tp_ps", bufs=3, space="PSUM") as tp_ps,
            tc.tile_pool(name="sc_ps", bufs=1, space="PSUM") as sc_ps,
            tc.tile_pool(name="g_ps", bufs=1, space="PSUM") as g_ps,
            tc.tile_pool(name="acc_ps", bufs=1, space="PSUM") as acc_ps,
        ):
            # ---- constants ----
            ident = consts.tile([P, P], BF16)
            make_identity(nc, ident[:])
            w_f = consts.tile([P, NCHUNK, H], F32)
            nc.sync.dma_start(w_f[:], W_temp.rearrange("(c p) h -> p c h", p=P))
            w_sb = consts.tile([P, NCHUNK, H], BF16)
            nc.vector.tensor_copy(w_sb[:], w_f[:])
            wf_sb = consts.tile([1, H], F32R)
            nc.sync.dma_start(wf_sb[:], W_fea[:])
            bw_sb = consts.tile([H, 1], F32)
            nc.sync.dma_start(bw_sb[:], bw[:, None])
            # uw_aug = [uw; 100.0] (mask fold: row 5 = raw mask 0/1; exp gets
            # bias=-100 so masked lanes underflow to zero)
            uwa_f = consts.tile([H + 1, 2], F32)
            nc.vector.memset(uwa_f[:], 100.0)
            nc.sync.dma_start(uwa_f[:H, 0:1], uw[:, None])
            nc.sync.dma_start(uwa_f[:H, 1:2], uw[:, None])
            uwa_sb = consts.tile([H + 1, 2], F32R)
            nc.vector.tensor_copy(uwa_sb[:], uwa_f[:])
            ones_sb = consts.tile([P, 1], BF16)
            nc.vector.memset(ones_sb[:], 1.0)
            nbias_sb = consts.tile([P, 1], F32)
            nc.vector.memset(nbias_sb[:], -100.0)
            # all mask rows up front: u8 -> f32 casting DMAs go to the SWDGE
            # queue BEFORE the big x3 stream floods it; ACT re-rounds to f32r
            mrow = consts.tile([1, b_shard * T], F32)
            mrowr = consts.tile([1, b_shard * T], F32R)
            for mb in range(b_shard):
                nc.gpsimd.dma_start(
                    mrow[0:1, mb * T : (mb + 1) * T], mask[mb : mb + 1, :]
                )
                nc.scalar.copy(
                    mrowr[0:1, mb * T : (mb + 1) * T], mrow[0:1, mb * T : (mb + 1) * T]
                )

            # pending num/den work for the previous group:
            # (xb, joff, g_sb, grp_idx, nm, den)
            pending = None
            tail = None        # deferred uw+exp emission for the previous group
            finalize = None    # deferred normalize/store for the previous batch
            n_pend = 3 * jg

            def emit_pending_item(pend, i):
                xb_, joff_, gsb_, g_, nm_, den_ = pend
                j = i // 3
                k = i % 3
                tt = g_ * jg + j
                first = tt == 0
                last = tt == n_tiles - 1
                if k < 2:
                    nc.tensor.matmul(
                        nm_[32:33, k * 512 : (k + 1) * 512],
                        gsb_[:, tt : tt + 1],
                        xb_[:, joff_ + j, k * 512 : (k + 1) * 512],
                        start=first,
                        stop=last,
                        tile_position=(0, 32),
                    )
                else:
                    nc.tensor.matmul(
                        den_[32:33, :],
                        gsb_[:, tt : tt + 1],
                        ones_sb[:],
                        start=first,
                        stop=last,
                        tile_position=(0, 32),
                    )

            for b in range(b_shard):
                # ---- per-batch rows ----
                fea_sb = rows.tile([1, T], F32R, tag="fea")
                nc.sync.dma_start(fea_sb[:], x_fea[b : b + 1, :])
                tanh_b = rows.tile([H + 1, T], F32R, tag="tanhb")
                # row 5 = raw mask (0/1), prepared up front
                nc.sync.dma_start(
                    tanh_b[H : H + 1, :], mrowr[0:1, b * T : (b + 1) * T]
                )
                g_sb = rows.tile([P, n_tiles], BF16, tag="gsb")

                nm = acc_ps.tile([33, D], F32, tag="num")
                den = acc_ps.tile([33, 1], F32, tag="den")

                for di in range(n_dma):
                    if use_dma_cast:
                        x3 = xpool.tile([P, jd, D], BF16, tag="x")
                        if b == 0 and di == 0:
                            # split the very first chunk so group 0's
                            # transposes start as soon as the first 512 t land
                            for s in range(jd // jg):
                                nc.gpsimd.dma_start(
                                    x3[:, s * jg : (s + 1) * jg, :],
                                    x_temp[
                                        b, s * t_grp : (s + 1) * t_grp, :
                                    ].rearrange("(j p) d -> p j d", p=P),
                                )
                        else:
                            nc.gpsimd.dma_start(
                                x3[:],
                                x_temp[
                                    b, di * dma_grp : (di + 1) * dma_grp, :
                                ].rearrange("(j p) d -> p j d", p=P),
                            )
                    else:
                        x3 = xpool.tile([P, jd, D], F32, tag="x")
                        nc.sync.dma_start(
                            x3[:],
                            x_temp[b, di * dma_grp : (di + 1) * dma_grp, :].rearrange(
                                "(j p) d -> p j d", p=P
                            ),
                        )
                    for gi in range(grp_per_dma):
                        g = di * grp_per_dma + gi
                        t0 = g * t_grp
                        if use_dma_cast:
                            xb = x3
                            joff = gi * jg
                        else:
                            xb = xbpool.tile([P, jg, D], BF16, tag="xb")
                            joff = 0
                            for j in range(jg):
                                if j % 4 == 3:
                                    nc.scalar.copy(xb[:, j, :], x3[:, gi * jg + j, :])
                                else:
                                    nc.vector.tensor_copy(
                                        xb[:, j, :], x3[:, gi * jg + j, :]
                                    )
                        # ---- transposes + DVE copies; prev group's uw+exp
                        # emitted between the two transpose halves ----
                        xts = []
                        for cp in range(NCHUNK // 2):
                            if cp == 2 and tail is not None:
                                tail()
                                tail = None
                            tp = tp_ps.tile([P, 2, t_grp], BF16, tag="tp")
                            for c2 in range(2):
                                c = cp * 2 + c2
                                for j in range(jg):
                                    nc.tensor.transpose(
                                        tp[:, c2, j * P : (j + 1) * P],
                                        xb[:, joff + j, c * P : (c + 1) * P],
                                        ident[:],
                                    )
                            xt = xtpool.tile([P, 2, t_grp], BF16, tag="xt")
                            nc.vector.tensor_copy(xt[:], tp[:])
                            xts.append(xt)
                        if tail is not None:
                            tail()
                            tail = None
                        # ---- scores stream (strip 0) with pending num/den
                        # (strip 1) interleaved => concurrent on the PE ----
                        sc = sc_ps.tile([H, t_grp], F32, tag="sc")
                        nc.tensor.matmul(
                            sc[:],
                            wf_sb[:],
                            fea_sb[:, t0 : t0 + t_grp],
                            start=True,
                            stop=False,
                        )
                        pi = 0
                        for c in range(NCHUNK):
                            if pending is not None:
                                take = 2 if c < (n_pend - NCHUNK) else 1
                                for _ in range(take):
                                    if pi < n_pend:
                                        emit_pending_item(pending, pi)
                                        pi += 1
                            nc.tensor.matmul(
                                sc[:],
                                w_sb[:, c, :],
                                xts[c // 2][:, c % 2, :],
                                start=False,
                                stop=(c == NCHUNK - 1),
                            )
                        if pending is not None:
                            while pi < n_pend:
                                emit_pending_item(pending, pi)
                                pi += 1
                            pending = None
                        # previous batch fully accumulated -> normalize/store
                        if finalize is not None:
                            finalize()
                            finalize = None
                        # tanh(sc + bw) -> tanh_b rows 0..4 (ACT, off PE path)
                        nc.scalar.activation(
                            tanh_b[:H, t0 : t0 + t_grp], sc[:], AF.Tanh, bias=bw_sb[:]
                        )

                        def make_tail(t0=t0, g=g, g_sb=g_sb, tanh_b=tanh_b):
                            def tail_fn():
                                gp = g_ps.tile([P, jg, 2], F32, tag="g")
                                for j in range(jg):
                                    nc.tensor.matmul(
                                        gp[:, j, :],
                                        tanh_b[:, t0 + j * P : t0 + (j + 1) * P],
                                        uwa_sb[:],
                                        start=True,
                                        stop=True,
                                    )
                                nc.scalar.activation(
                                    g_sb[:, g * jg : (g + 1) * jg],
                                    gp[:, :, 0],
                                    AF.Exp,
                                    bias=nbias_sb[:],
                                )
                            return tail_fn

                        tail = make_tail()
                        pending = (xb, joff, g_sb, g, nm, den)

                # defer normalize/store: pending of the last group drains in
                # the next batch's first scores stream
                def make_finalize(b=b, nm=nm, den=den):
                    def fin():
                        inv = small.tile([33, 1], F32, tag="inv")
                        nc.vector.reciprocal(inv[32:33, :], den[32:33, :])
                        o_sb = small.tile([33, D], F32, tag="osb")
                        nc.vector.tensor_scalar_mul(
                            o_sb[32:33, :], nm[32:33, :], inv[32:33, :]
                        )
                        nc.sync.dma_start(out[b : b + 1, :], o_sb[32:33, :])
                    return fin

                finalize = make_finalize()

            # drain the last batch
            if tail is not None:
                tail()
                tail = None
            if pending is not None:
                for i in range(n_pend):
                    emit_pending_item(pending, i)
                pending = None
            if finalize is not None:
                finalize()
                finalize = None

    nc.finalize()
    return nc


_NC_CACHE = {}


def _get_nc(b_shard, T):
    key = (b_shard, T)
    if key not in _NC_CACHE:
        _NC_CACHE[key] = build_kernel(b_shard, T)
    return _NC_CACHE[key]


def kernel(x_temp, x_fea, mask, W_temp, W_fea, bw, uw) -> np.ndarray:
    from concourse.bass_utils import run_bass_kernel_spmd

    B, T, D_ = x_temp.shape
    n_cores = 8
    assert B % n_cores == 0
    bs = B // n_cores

    nc = _get_nc(bs, T)

    x_temp = np.ascontiguousarray(x_temp, dtype=np.float32)
    x_fea = np.ascontiguousarray(x_fea, dtype=np.float32)
    mask_u8 = np.ascontiguousarray(mask).view(np.uint8)
    W_temp = np.ascontiguousarray(W_temp, dtype=np.float32)
    W_fea = np.ascontiguousarray(W_fea, dtype=np.float32)
    bw = np.ascontiguousarray(bw, dtype=np.float32)
    uw = np.ascontiguousarray(uw, dtype=np.float32)

    in_maps = []
    for i in range(n_cores):
        in_maps.append(
            {
                "x_temp": x_temp[i * bs : (i + 1) * bs],
                "x_fea": x_fea[i * bs : (i + 1) * bs],
                "mask": mask_u8[i * bs : (i + 1) * bs],
                "W_temp": W_temp,
                "W_fea": W_fea,
                "bw": bw,
                "uw": uw,
            }
        )

    res = run_bass_kernel_spmd(nc, in_maps, core_ids=list(range(n_cores)))
    return np.concatenate([r["out"] for r in res.results], axis=0)
